# revision 17
# baseline (speedup 1.0000x reference)
"""Trainium2 Bass kernel for nn_MessagePassingLayer (GNN message passing).

Strategy (edge-parallel, col-sharded, 8 cores, no collectives):
  - Host sorts edges by (destination window, src-row-half, src row) and splits
    them into 8 contiguous col ranges (aligned to node boundaries), so each
    core owns a node range and all edges targeting it.  Scatter-add never
    crosses cores.
  - Algebraic refactor: msg = relu(x[row]@Wm1a + x[col]@Wm1b + ea@Wm1c + bm1)
    aggr = segsum(msg@Wm2 + bm2) = segsum(H)@Wm2 + deg*bm2, so the per-edge
    128x128 matmul Wm2 moves to the node side:
      P[n]  = segsum_n relu(A[row] + B[col] + C_e + bm1)   (A=x@Wm1a, B=x@Wm1b)
      out   = relu(x@Wu1a + P@(Wm2@Wu1b) + deg*(bm2@Wu1b) + bu1) @ Wu2 + bu2
  - Device: pre-phase computes bf16 A (all nodes, split at row 32768 into two
    tables for int16 gather indices) and B (local nodes) tables in DRAM.
    Edge phase gathers A[row] / B[col] rows for GB=4 windows per dma_gather
    (the MoE bulk-gather custom instruction: thousands of 256B rows per
    gpsimd op).  Per 4-tile group the C contribution comes from K=33 matmuls
    of edge_attr^T; the gathered A and B tiles accumulate into the same PSUM
    via identity matmuls; relu evacuates on the scalar engine; one-hot Sel is
    built 4 tiles at a time on DVE in bf16; the scatter matmul emits the
    aggregate directly feature-major (P^T[f,n] = h^T@sel).
  - Update phase consumes P^T in place (no transposes); bu1/bu2 fold into
    activation biases; output is stored feature-major, transposed on host.
  - Edges are packed per 128-node window with lo/hi split padding
    (colshift=-1 pads have all-zero Sel rows, contributing nothing) so the
    program is SPMD-uniform.
"""
import sys
sys.path.insert(0, '/opt/trn_rl_repo')

import os
import time
import numpy as np
import ml_dtypes

BF16 = ml_dtypes.bfloat16

N_CORES = 8
P = 128
NODE_IN = 128
EDGE_IN = 32
OUT_DIM = 128
GB = 4        # windows per batched gather
SPLIT = 32768  # int16 gather index range

_DEBUG = bool(int(os.environ.get("K_DEBUG", "0")))


_GRP = int(os.environ.get("K_GRP", "4"))


def _chunks(n, c=None):
    c = _GRP if c is None else c
    return [c] * (n // c) + ([n % c] if n % c else [])


# ---------------------------------------------------------------- host prep

def _host_prep(x, edge_index, edge_attr):
    x = np.asarray(x, np.float32)
    n_nodes = x.shape[0]
    npad = -(-n_nodes // P) * P
    row = np.asarray(edge_index[0], dtype=np.int64)
    col = np.asarray(edge_index[1], dtype=np.int64)
    perm = np.argsort(col, kind='stable')
    row_s = row[perm].astype(np.int32)
    col_s = col[perm].astype(np.int32)
    ea_s = np.asarray(edge_attr, dtype=np.float32)[perm]

    E = row_s.shape[0]
    cuts = (np.arange(1, N_CORES) * E) // N_CORES
    nb = [0] + [int(col_s[c]) for c in cuts] + [n_nodes]
    for i in range(1, len(nb)):  # enforce nondecreasing
        nb[i] = max(nb[i], nb[i - 1])
    lo = [int(np.searchsorted(col_s, nb[k])) for k in range(N_CORES)] + [E]
    n_k = [nb[k + 1] - nb[k] for k in range(N_CORES)]
    N_LOC = max(128, int(-(-max(n_k) // 128)) * 128)
    W = N_LOC // 128
    assert N_LOC < SPLIT

    # pass 1: per-(core, window, half) fill counts -> global T_LO / T_HI
    T_LO = T_HI = 0
    percore = []
    for k in range(N_CORES):
        l, h = lo[k], lo[k + 1]
        colrel = col_s[l:h] - nb[k]
        rows = row_s[l:h]
        wins = colrel >> 7
        hif = (rows >= SPLIT)
        order = np.lexsort((rows, hif, wins))
        r_o, c_o, w_o, h_o = rows[order], colrel[order], wins[order], hif[order]
        cnt_lo = np.bincount(w_o[~h_o], minlength=W)
        cnt_hi = np.bincount(w_o[h_o], minlength=W)
        if cnt_lo.max(initial=0) > 0:
            T_LO = max(T_LO, int(-(-cnt_lo.max() // 128)))
        if cnt_hi.max(initial=0) > 0:
            T_HI = max(T_HI, int(-(-cnt_hi.max() // 128)))
        percore.append((l, h, order, r_o, c_o, w_o, h_o))
    T = T_LO + T_HI
    assert T >= 1
    E_LOC = W * T * 128
    WT = W * T

    cores = []
    for k in range(N_CORES):
        l, h, order, r_o, c_o, w_o, h_o = percore[k]
        ne = h - l
        colshift = np.full(E_LOC, -1.0, np.float32)
        eaT = np.zeros((EDGE_IN + 1, E_LOC), BF16)
        eaT[EDGE_IN, :] = 1.0
        idxAlo = np.zeros(W * max(T_LO, 1) * 128, np.int16)
        idxAhi = np.zeros(W * max(T_HI, 1) * 128, np.int16)
        idxB = np.zeros(E_LOC, np.int16)
        deg = np.zeros(N_LOC, np.float32)
        if ne > 0:
            run = w_o.astype(np.int64) * 2 + h_o
            run_sizes = np.bincount(run, minlength=2 * W)
            run_start = np.zeros(2 * W, np.int64)
            run_start[1:] = np.cumsum(run_sizes)[:-1]
            within = np.arange(ne, dtype=np.int64) - run_start[run]
            slot = w_o.astype(np.int64) * (T * 128) + np.where(
                h_o, T_LO * 128 + within, within)
            colshift[slot] = (c_o - (w_o << 7)).astype(np.float32)
            eaT[:EDGE_IN, slot] = np.asarray(ea_s[l:h], np.float32)[order].T.astype(BF16)
            idxB[slot] = c_o.astype(np.int16)
            mlo = ~h_o
            if mlo.any():
                pos = w_o[mlo].astype(np.int64) * (T_LO * 128) + within[mlo]
                idxAlo[pos] = r_o[mlo].astype(np.int16)
            if h_o.any():
                pos = w_o[h_o].astype(np.int64) * (T_HI * 128) + within[h_o]
                idxAhi[pos] = (r_o[h_o] - SPLIT).astype(np.int16)
            deg[:n_k[k]] = np.bincount(c_o, minlength=n_k[k]).astype(np.float32)[:n_k[k]]

        def wrap_batches(flat, tpw):
            if tpw == 0:
                return None
            segs = []
            for w0 in range(0, W, GB):
                g = min(GB, W - w0)
                seg = flat[w0 * tpw * 128:(w0 + g) * tpw * 128]
                segs.append(np.tile(seg.reshape(-1, 16).T, (8, 1)))
            return np.ascontiguousarray(np.concatenate(segs, axis=1))

        xT_loc = np.zeros((P, N_LOC), np.float32)
        xT_loc[:, :n_k[k]] = x.T[:, nb[k]:nb[k] + n_k[k]]
        cores.append({
            "colshift": np.ascontiguousarray(
                colshift.reshape(WT, 128).T.astype(BF16)),
            "idxAlo": wrap_batches(idxAlo, T_LO),
            "idxAhi": wrap_batches(idxAhi, T_HI),
            "idxB": wrap_batches(idxB, T),
            "eaT": eaT,
            "degT": deg.reshape(1, N_LOC),
            "xTlocbf": xT_loc.astype(BF16),
        })
    xTbf = np.zeros((P, npad), BF16)
    xTbf[:, :n_nodes] = x.T.astype(BF16)
    cfg = {"N_LOC": N_LOC, "W": W, "T": T, "T_LO": T_LO, "T_HI": T_HI,
           "E_LOC": E_LOC, "nb": nb, "n_k": n_k, "NPAD": npad,
           "N_NODES": n_nodes}
    return cfg, xTbf, cores


def _host_weights(Wm1, bm1, Wm2, bm2, Wu1, bu1, Wu2, bu2):
    Wm1 = np.asarray(Wm1, np.float32)
    Wu1 = np.asarray(Wu1, np.float32)
    w = {}
    w["wm1ab"] = np.concatenate([Wm1[0:128], Wm1[128:256]], axis=1).astype(BF16)
    w["wm1c"] = np.concatenate([Wm1[256:256 + EDGE_IN],
                                np.asarray(bm1, np.float32)[None, :]],
                               axis=0).astype(BF16)                  # [33,128]
    w["wpb"] = np.asarray(Wm2, np.float32) @ Wu1[128:256]            # [128,128]
    w["wu1a"] = np.ascontiguousarray(Wu1[0:128]).astype(BF16)        # [128,128]
    w["wu2"] = np.asarray(Wu2, np.float32)                           # [128,128]
    vecs = np.zeros((1, 128), np.float32)
    vecs[0] = np.asarray(bm2, np.float32) @ Wu1[128:256]             # vb
    w["vecs"] = vecs
    bcols = np.zeros((128, 2), np.float32)
    bcols[:, 0] = np.asarray(bu1, np.float32)
    bcols[:, 1] = np.asarray(bu2, np.float32)
    w["bcols"] = bcols
    return w


# ---------------------------------------------------------------- bass build

def _build(cfg):
    import concourse.bass as bass
    import concourse.mybir as mybir
    from concourse import bacc, tile
    from concourse.masks import make_identity

    f32 = mybir.dt.float32
    bf16 = mybir.dt.bfloat16
    i16 = mybir.dt.int16
    i32 = mybir.dt.int32
    Alu = mybir.AluOpType
    Act = mybir.ActivationFunctionType

    N_LOC, W, T, E_LOC = cfg["N_LOC"], cfg["W"], cfg["T"], cfg["E_LOC"]
    T_LO, T_HI = cfg["T_LO"], cfg["T_HI"]
    NPAD = cfg["NPAD"]
    WT = W * T
    EA = EDGE_IN + 1
    NA1 = min(NPAD, SPLIT)       # rows in A table (low half)
    NA2 = NPAD - NA1             # rows in A2 (high half)

    nc = bacc.Bacc("TRN2", target_bir_lowering=False, debug=False,
                   num_devices=N_CORES)

    xTbf_d = nc.dram_tensor("xTbf", [P, NPAD], bf16, kind="ExternalInput")
    xTlocbf_d = nc.dram_tensor("xTlocbf", [P, N_LOC], bf16, kind="ExternalInput")
    eaT_d = nc.dram_tensor("eaT", [EA, E_LOC], bf16, kind="ExternalInput")
    colshift_d = nc.dram_tensor("colshift", [P, WT], bf16, kind="ExternalInput")
    SA_LO = W * T_LO * 8
    SA_HI = W * T_HI * 8
    SB = W * T * 8
    idxAlo_d = (nc.dram_tensor("idxAlo", [P, SA_LO], i16, kind="ExternalInput")
                if T_LO else None)
    idxAhi_d = (nc.dram_tensor("idxAhi", [P, SA_HI], i16, kind="ExternalInput")
                if T_HI else None)
    idxB_d = nc.dram_tensor("idxB", [P, SB], i16, kind="ExternalInput")
    degT_d = nc.dram_tensor("degT", [1, N_LOC], f32, kind="ExternalInput")
    wm1ab_d = nc.dram_tensor("wm1ab", [P, 256], bf16, kind="ExternalInput")
    wm1c_d = nc.dram_tensor("wm1c", [EA, P], bf16, kind="ExternalInput")
    wpb_d = nc.dram_tensor("wpb", [P, P], f32, kind="ExternalInput")
    wu1a_d = nc.dram_tensor("wu1a", [P, P], bf16, kind="ExternalInput")
    wu2_d = nc.dram_tensor("wu2", [P, P], f32, kind="ExternalInput")
    vecs_d = nc.dram_tensor("vecs", [1, P], f32, kind="ExternalInput")
    bcols_d = nc.dram_tensor("bcols", [P, 2], f32, kind="ExternalInput")
    out_d = nc.dram_tensor("out", [P, N_LOC], f32, kind="ExternalOutput")

    A_d = nc.dram_tensor("Atbl", [NA1, P], bf16, kind="Internal")
    A2_d = nc.dram_tensor("Atbl2", [max(NA2, P), P], bf16, kind="Internal") \
        if NA2 else None
    B_d = nc.dram_tensor("Btbl", [N_LOC, P], bf16, kind="Internal")
    if _DEBUG:
        dbgP_d = nc.dram_tensor("dbgP", [P, N_LOC], f32, kind="ExternalOutput")

    with tile.TileContext(nc) as tc:
        with tc.tile_pool(name="const", bufs=1) as cp, \
             tc.tile_pool(name="res", bufs=1) as rp:
            wm1ab = cp.tile([P, 256], bf16)
            nc.sync.dma_start(out=wm1ab[:], in_=wm1ab_d[:])
            wm1c = cp.tile([EA, P], bf16)
            nc.sync.dma_start(out=wm1c[:], in_=wm1c_d[:])
            wpb = cp.tile([P, P], f32)
            nc.sync.dma_start(out=wpb[:], in_=wpb_d[:])
            wu1a = cp.tile([P, P], bf16)
            nc.sync.dma_start(out=wu1a[:], in_=wu1a_d[:])
            wu2 = cp.tile([P, P], f32)
            nc.sync.dma_start(out=wu2[:], in_=wu2_d[:])
            vb_sb = cp.tile([1, P], f32)
            nc.sync.dma_start(out=vb_sb[:], in_=vecs_d[0:1, :])
            bcols = cp.tile([P, 2], f32)
            nc.sync.dma_start(out=bcols[:], in_=bcols_d[:])
            ident_bf = cp.tile([P, P], bf16)
            make_identity(nc, ident_bf[:])
            iota_i = cp.tile([P, P], i32)
            nc.gpsimd.iota(iota_i[:], [[1, P]], channel_multiplier=0)
            iota4 = cp.tile([P, 4 * P], bf16)
            for j in range(4):
                nc.vector.tensor_copy(out=iota4[:, j * P:(j + 1) * P], in_=iota_i[:])

            colshift = rp.tile([P, WT], bf16)
            nc.sync.dma_start(out=colshift[:], in_=colshift_d[:])
            xTlocbf = rp.tile([P, N_LOC], bf16)
            nc.sync.dma_start(out=xTlocbf[:], in_=xTlocbf_d[:])
            degT = rp.tile([1, N_LOC], f32)
            nc.sync.dma_start(out=degT[:], in_=degT_d[:])
            if T_LO:
                idxAlo = rp.tile([P, SA_LO], i16)
                nc.sync.dma_start(out=idxAlo[:], in_=idxAlo_d[:])
            if T_HI:
                idxAhi = rp.tile([P, SA_HI], i16)
                nc.sync.dma_start(out=idxAhi[:], in_=idxAhi_d[:])
            idxB = rp.tile([P, SB], i16)
            nc.sync.dma_start(out=idxB[:], in_=idxB_d[:])
            P_loc = rp.tile([P, N_LOC], f32)

            # ---------------- pre phase: A (all nodes), B (local), bf16
            # batched stores: m row-tiles per DMA via 3D dst AP (p, j, f)
            def store_rows(tbl, r0, ev, m):
                nc.sync.dma_start(
                    out=tbl[r0:r0 + m * P, :].rearrange("(j p) f -> p j f", p=P),
                    in_=ev[:, 0:m * P])

            CH = 8  # tiles per chunk (SPLIT//P == 256 divides by CH)
            with tc.tile_pool(name="pre_sb", bufs=3) as pp, \
                 tc.tile_pool(name="pre_ps", bufs=4, space="PSUM") as pps:
                for i0 in range(0, NPAD // P, CH):
                    m = min(CH, NPAD // P - i0)
                    xt = pp.tile([P, CH * P], bf16, tag="xt")
                    nc.sync.dma_start(out=xt[:, 0:m * P],
                                      in_=xTbf_d[:, i0 * P:(i0 + m) * P])
                    ev = pp.tile([P, CH * P], bf16, tag="ev")
                    for h0 in range(0, m, 4):
                        hm = min(4, m - h0)
                        ps = pps.tile([P, 4 * P], f32, tag="ps")
                        for j in range(hm):
                            nc.tensor.matmul(out=ps[:, j * P:(j + 1) * P],
                                             lhsT=xt[:, (h0 + j) * P:(h0 + j + 1) * P],
                                             rhs=wm1ab[:, 0:P], start=True, stop=True)
                        nc.vector.tensor_copy(out=ev[:, h0 * P:(h0 + hm) * P],
                                              in_=ps[:, 0:hm * P])
                    if i0 * P >= NA1:
                        store_rows(A2_d, i0 * P - NA1, ev, m)
                    elif (i0 + m) * P <= NA1:
                        store_rows(A_d, i0 * P, ev, m)
                    else:
                        mlo = (NA1 - i0 * P) // P
                        store_rows(A_d, i0 * P, ev, mlo)
                        nc.sync.dma_start(
                            out=A2_d[0:(m - mlo) * P, :].rearrange(
                                "(j p) f -> p j f", p=P),
                            in_=ev[:, mlo * P:m * P])
                for i0 in range(0, W, 4):
                    m = min(4, W - i0)
                    ps = pps.tile([P, 4 * P], f32, tag="ps")
                    for j in range(m):
                        nc.tensor.matmul(out=ps[:, j * P:(j + 1) * P],
                                         lhsT=xTlocbf[:, (i0 + j) * P:(i0 + j + 1) * P],
                                         rhs=wm1ab[:, P:2 * P], start=True, stop=True)
                    ev = pp.tile([P, CH * P], bf16, tag="ev")
                    nc.vector.tensor_copy(out=ev[:, 0:m * P], in_=ps[:, 0:m * P])
                    store_rows(B_d, i0 * P, ev, m)

            # ---------------- edge phase
            pools = []
            glo = _chunks(T_LO)
            ghi = _chunks(T_HI)
            with tc.tile_pool(name="eg_hlo", bufs=2) as hlo_p, \
                 tc.tile_pool(name="eg_hhi", bufs=2) as hhi_p, \
                 tc.tile_pool(name="eg_hb", bufs=2) as hb_p, \
                 tc.tile_pool(name="eg_ea", bufs=2) as eap, \
                 tc.tile_pool(name="eg_sb", bufs=4) as ep, \
                 tc.tile_pool(name="eg_ps", bufs=3, space="PSUM") as cps, \
                 tc.tile_pool(name="p_ps", bufs=2, space="PSUM") as pps2:
                for w0 in range(0, W, GB):
                    g = min(GB, W - w0)
                    hA_lo = hA_hi = None
                    if T_LO:
                        n = g * T_LO * 128
                        hA_lo = hlo_p.tile([P, GB * T_LO * P], bf16, tag="hlo")
                        nc.gpsimd.dma_gather(
                            hA_lo[:, 0:n].rearrange("p (t f) -> p t f", f=P),
                            A_d[:],
                            idxAlo[:, w0 * T_LO * 8:(w0 + g) * T_LO * 8],
                            n, n, P, single_packet=False)
                    if T_HI:
                        n = g * T_HI * 128
                        hA_hi = hhi_p.tile([P, GB * T_HI * P], bf16, tag="hhi")
                        nc.gpsimd.dma_gather(
                            hA_hi[:, 0:n].rearrange("p (t f) -> p t f", f=P),
                            A2_d[:],
                            idxAhi[:, w0 * T_HI * 8:(w0 + g) * T_HI * 8],
                            n, n, P, single_packet=False)
                    n = g * T * 128
                    hB = hb_p.tile([P, GB * T * P], bf16, tag="hb")
                    nc.gpsimd.dma_gather(
                        hB[:, 0:n].rearrange("p (t f) -> p t f", f=P),
                        B_d[:],
                        idxB[:, w0 * T * 8:(w0 + g) * T * 8],
                        n, n, P, single_packet=False)
                    ea_sb = eap.tile([EA, GB * T * P], bf16, tag="ea")
                    nc.sync.dma_start(out=ea_sb[:, 0:g * T * P],
                                      in_=eaT_d[:, w0 * T * P:(w0 + g) * T * P])
                    for wl in range(g):
                        w = w0 + wl
                        psP = pps2.tile([P, P], f32, tag="psP")
                        for half, gr, tpw, hbuf in (
                                (0, glo, T_LO, hA_lo), (1, ghi, T_HI, hA_hi)):
                            t0h = 0
                            for s in gr:
                                tg0 = (T_LO if half else 0) + t0h  # global tile
                                bt = wl * T + tg0                  # ea/colshift col
                                psC = cps.tile([P, 4 * P], f32, tag="psC")
                                ha = (wl * tpw + t0h) * P
                                hb0 = (wl * T + tg0) * P
                                # one accumulation group open per PSUM bank at
                                # a time: complete each region's chain before
                                # opening the next
                                for j in range(s):
                                    sl_ps = slice(j * P, (j + 1) * P)
                                    nc.tensor.matmul(
                                        out=psC[:, sl_ps],
                                        lhsT=ea_sb[:, (bt + j) * P:(bt + j + 1) * P],
                                        rhs=wm1c[:], start=True, stop=False)
                                    nc.tensor.matmul(
                                        out=psC[:, sl_ps], lhsT=ident_bf[:],
                                        rhs=hbuf[:, ha + j * P:ha + (j + 1) * P],
                                        start=False, stop=False)
                                    nc.tensor.matmul(
                                        out=psC[:, sl_ps], lhsT=ident_bf[:],
                                        rhs=hB[:, hb0 + j * P:hb0 + (j + 1) * P],
                                        start=False, stop=True)
                                hr = ep.tile([P, 4 * P], bf16, tag="hr")
                                nc.scalar.activation(out=hr[:, 0:s * P],
                                                     in_=psC[:, 0:s * P],
                                                     func=Act.Relu)
                                sel = ep.tile([P, 4 * P], bf16, tag="sel")
                                tcol = w * T + tg0
                                nc.vector.tensor_tensor(
                                    out=sel[:, 0:s * P],
                                    in0=iota4[:, 0:s * P],
                                    in1=colshift[:, tcol:tcol + s].unsqueeze(2)
                                        .to_broadcast([P, s, P]),
                                    op=Alu.is_equal)
                                for j in range(s):
                                    t = tg0 + j
                                    nc.tensor.matmul(
                                        out=psP[:],
                                        lhsT=hr[:, j * P:(j + 1) * P],
                                        rhs=sel[:, j * P:(j + 1) * P],
                                        start=(t == 0), stop=(t == T - 1))
                                t0h += s
                        nc.scalar.copy(out=P_loc[:, w * P:(w + 1) * P], in_=psP[:])

            if _DEBUG:
                nc.sync.dma_start(out=dbgP_d[:], in_=P_loc[:])

            # ---------------- update phase (feature-major throughout)
            with tc.tile_pool(name="up_sb", bufs=3) as up, \
                 tc.tile_pool(name="upA_ps", bufs=2, space="PSUM") as upsA, \
                 tc.tile_pool(name="upB_ps", bufs=2, space="PSUM") as upsB:
                for w0 in range(0, W, 4):
                    g = min(4, W - w0)
                    o2s = up.tile([P, 4 * P], f32, tag="o2s")
                    for wl in range(g):
                        w = w0 + wl
                        sl = slice(w * P, (w + 1) * P)
                        ut = upsA.tile([P, P], f32, tag="ut")
                        nc.tensor.matmul(out=ut[:], lhsT=wpb[:], rhs=P_loc[:, sl],
                                         start=True, stop=False)
                        nc.tensor.matmul(out=ut[:], lhsT=wu1a[:], rhs=xTlocbf[:, sl],
                                         start=False, stop=False)
                        nc.tensor.matmul(out=ut[:], lhsT=vb_sb[:], rhs=degT[:, sl],
                                         start=False, stop=True)
                        r = up.tile([P, P], f32, tag="r")
                        nc.scalar.activation(out=r[:], in_=ut[:], func=Act.Relu,
                                             bias=bcols[:, 0:1])
                        o2 = upsB.tile([P, P], f32, tag="o2")
                        nc.tensor.matmul(out=o2[:], lhsT=wu2[:], rhs=r[:],
                                         start=True, stop=True)
                        nc.scalar.activation(out=o2s[:, wl * P:(wl + 1) * P],
                                             in_=o2[:],
                                             func=Act.Identity, bias=bcols[:, 1:2])
                    nc.sync.dma_start(out=out_d[:, w0 * P:(w0 + g) * P],
                                      in_=o2s[:, 0:g * P])

    nc.compile()
    return nc


# ---------------------------------------------------------------- runner

class SpmdRunner:
    """Jit-once PJRT runner for a prebuilt Bass module (axon path)."""

    def __init__(self, nc, n_cores):
        import jax
        from jax.sharding import Mesh, PartitionSpec
        from jax.experimental.shard_map import shard_map
        import concourse.mybir as mybir
        from concourse import bass2jax
        from concourse.bass2jax import _bass_exec_p, install_neuronx_cc_hook

        install_neuronx_cc_hook()
        self.jax = jax
        self.nc = nc
        self.n_cores = n_cores
        partition_name = nc.partition_id_tensor.name if nc.partition_id_tensor else None
        in_names, out_names, out_avals = [], [], []
        for alloc in nc.m.functions[0].allocations:
            if not isinstance(alloc, mybir.MemoryLocationSet):
                continue
            name = alloc.memorylocations[0].name
            if alloc.kind == "ExternalInput":
                if name != partition_name:
                    in_names.append(name)
            elif alloc.kind == "ExternalOutput":
                out_names.append(name)
                out_avals.append(jax.core.ShapedArray(
                    tuple(alloc.tensor_shape), mybir.dt.np(alloc.dtype)))
        self.in_names, self.out_names, self.out_avals = in_names, out_names, out_avals
        bind_in_names = list(in_names) + list(out_names)
        if partition_name is not None:
            bind_in_names.append(partition_name)
        n_params = len(in_names)
        n_outs = len(out_names)
        self._n_params, self._n_outs = n_params, n_outs

        def _exec_once(ins, outs):
            operands = list(ins) + list(outs)
            if partition_name is not None:
                operands.append(bass2jax.partition_id_tensor())
            return _bass_exec_p.bind(
                *operands,
                out_avals=tuple(out_avals),
                in_names=tuple(bind_in_names),
                out_names=tuple(out_names),
                lowering_input_output_aliases=(),
                sim_require_finite=True,
                sim_require_nnan=True,
                nc=nc,
            )

        self._exec_once = _exec_once

        def _body(*args):
            return tuple(_exec_once(args[:n_params], args[n_params:]))

        devices = jax.devices()[:n_cores]
        assert len(devices) == n_cores
        donate = tuple(range(n_params, n_params + n_outs))
        self._sharding = None
        if n_cores == 1:
            self._fn = jax.jit(_body, donate_argnums=donate, keep_unused=True)
            self._concat = False
            self._wrap = lambda f: jax.jit(f, keep_unused=True)
        else:
            from jax.sharding import NamedSharding
            mesh = Mesh(np.asarray(devices), ("core",))
            self._sharding = NamedSharding(mesh, PartitionSpec("core"))

            def _wrap(f):
                return jax.jit(shard_map(
                    f, mesh=mesh,
                    in_specs=(PartitionSpec("core"),) * (n_params + n_outs),
                    out_specs=(PartitionSpec("core"),) * n_outs,
                    check_rep=False), keep_unused=True)
            self._wrap = _wrap
            self._fn = jax.jit(shard_map(
                _body, mesh=mesh,
                in_specs=(PartitionSpec("core"),) * (n_params + n_outs),
                out_specs=(PartitionSpec("core"),) * n_outs,
                check_rep=False), donate_argnums=donate, keep_unused=True)
            self._concat = True

        import jax.numpy as jnp

        def _mk():
            outs = []
            for av in self.out_avals:
                shape = (self.n_cores * av.shape[0], *av.shape[1:]) if n_cores > 1 else av.shape
                outs.append(jnp.zeros(shape, av.dtype))
            return tuple(outs)

        if self._sharding is not None:
            self._mkzeros = jax.jit(
                _mk, out_shardings=tuple(self._sharding for _ in out_names))
        else:
            self._mkzeros = jax.jit(_mk)
        self._args = None
        self._chain_fns = {}

    def set_inputs(self, in_maps):
        assert len(in_maps) == self.n_cores
        args = []
        for name in self.in_names:
            if self._concat:
                args.append(np.concatenate(
                    [in_maps[c][name] for c in range(self.n_cores)], axis=0))
            else:
                args.append(in_maps[0][name])
        if self._sharding is not None:
            self._args = [self.jax.device_put(a, self._sharding) for a in args]
        else:
            self._args = [self.jax.device_put(a) for a in args]
        self.jax.block_until_ready(self._args)

    def run(self):
        outs = self._fn(*self._args, *self._mkzeros())
        self.jax.block_until_ready(outs)
        return outs

    def results(self, outs):
        res = []
        for c in range(self.n_cores):
            d = {}
            for i, name in enumerate(self.out_names):
                a = np.asarray(outs[i])
                if self._concat:
                    a = a.reshape(self.n_cores, *self.out_avals[i].shape)[c]
                d[name] = a
            res.append(d)
        return res

    def time(self, iters=30, warmup=3):
        ts = []
        for _ in range(warmup):
            self.run()
        for _ in range(iters):
            t0 = time.perf_counter()
            self.run()
            ts.append(time.perf_counter() - t0)
        return float(np.median(ts)), float(np.min(ts))

    # -- chained timing: one dispatch runs the kernel k times back-to-back,
    #    serialized through the output buffers; slope over k isolates the
    #    per-iteration device time from the ~90ms axon dispatch overhead.
    def _chain(self, k):
        if k not in self._chain_fns:
            n_params = self._n_params

            def _body_k(*args):
                ins = args[:n_params]
                outs = tuple(args[n_params:])
                for _ in range(k):
                    outs = tuple(self._exec_once(ins, outs))
                return outs

            self._chain_fns[k] = self._wrap(_body_k)
        return self._chain_fns[k]

    def time_slope(self, k_lo=2, k_hi=10, iters=6, warmup=2):
        f_lo, f_hi = self._chain(k_lo), self._chain(k_hi)
        zeros = self._mkzeros()
        walls = {}
        for k, f in ((k_lo, f_lo), (k_hi, f_hi)):
            for _ in range(warmup):
                self.jax.block_until_ready(f(*self._args, *zeros))
            ts = []
            for _ in range(iters):
                t0 = time.perf_counter()
                self.jax.block_until_ready(f(*self._args, *zeros))
                ts.append(time.perf_counter() - t0)
            walls[k] = ts
        med = (np.median(walls[k_hi]) - np.median(walls[k_lo])) / (k_hi - k_lo)
        mn = (np.min(walls[k_hi]) - np.min(walls[k_lo])) / (k_hi - k_lo)
        return float(med), float(mn), walls


# ---------------------------------------------------------------- entry

_CACHE = {}


def _get_runner(cfg):
    key = (cfg["N_LOC"], cfg["W"], cfg["T"], cfg["T_LO"], cfg["T_HI"],
           cfg["NPAD"], _DEBUG, _GRP)
    if key not in _CACHE:
        nc = _build(cfg)
        _CACHE[key] = SpmdRunner(nc, N_CORES)
    return _CACHE[key]


def _make_in_maps(cfg, xTbf, cores, w):
    in_maps = []
    for k in range(N_CORES):
        c = cores[k]
        m = {
            "xTbf": xTbf, "xTlocbf": c["xTlocbf"], "eaT": c["eaT"],
            "colshift": c["colshift"], "idxB": c["idxB"], "degT": c["degT"],
            "wm1ab": w["wm1ab"], "wm1c": w["wm1c"], "wpb": w["wpb"],
            "wu1a": w["wu1a"], "wu2": w["wu2"], "vecs": w["vecs"],
            "bcols": w["bcols"],
        }
        if cfg["T_LO"]:
            m["idxAlo"] = c["idxAlo"]
        if cfg["T_HI"]:
            m["idxAhi"] = c["idxAhi"]
        in_maps.append(m)
    return in_maps


def _assemble(cfg, results):
    full = np.empty((cfg["N_NODES"], OUT_DIM), np.float32)
    nb, n_k = cfg["nb"], cfg["n_k"]
    for k in range(N_CORES):
        if n_k[k] > 0:
            full[nb[k]:nb[k] + n_k[k]] = results[k]["out"][:, :n_k[k]].T
    return full


def kernel(**inputs):
    x = np.asarray(inputs["x"], np.float32)
    cfg, xTbf, cores = _host_prep(x, inputs["edge_index"], inputs["edge_attr"])
    w = _host_weights(inputs["Wm1"], inputs["bm1"], inputs["Wm2"], inputs["bm2"],
                      inputs["Wu1"], inputs["bu1"], inputs["Wu2"], inputs["bu2"])
    runner = _get_runner(cfg)
    runner.set_inputs(_make_in_maps(cfg, xTbf, cores, w))
    outs = runner.run()
    return _assemble(cfg, runner.results(outs))
